# revision 35
# baseline (speedup 1.0000x reference)
"""Causal self-attention (B=2, T=2048, C=1024, H=16, D=64) on 8 NeuronCores.

Sharding: core = (batch b, head-group g); each of the 8 cores handles one
batch and 4 of the 16 heads (data parallel on B, tensor parallel on heads).
Each core computes q/k/v projections for its heads, rope, causal softmax
attention, and a partial out-projection; the host sums the 4 per-batch
partials and adds bout + bqkv_v @ Wout (the v-bias term commutes through
the attention average, so it is applied once on the host).

v2 design (vs f32r baseline): everything bf16 on the PE and elementwise
engines, fp32 only in PSUM accumulators and the final y partial.
  - QT=256 so each PSUM tile is exactly one 2KB bank: S tiles (pss, 3 bufs)
    + O accumulators (psot, 2 bufs) + projection tiles (proj, 3 bufs) = 8
    banks, eliminating the pool contention that stalled the PE and caused
    HAM clock oscillation in the baseline.
  - ScalarE does (almost) only the softmax exp; q/k bias-add is folded into
    the PSUM evacuation on DVE (tensor_scalar_add with per-partition bias);
    out-proj v-bias moved to host.
  - v is padded to 66 cols/head (col 64 = ones for the softmax denominator,
    col 65 = zero pad for 4-byte alignment of bf16 lhsT slices).
  - a dummy exp at t=0 pulls the ~2.7us ACT table load into the DMA
    prologue.
"""
import sys
sys.path.insert(0, '/opt/trn_rl_repo')

import numpy as np
import ml_dtypes
from contextlib import ExitStack

import concourse.bass as bass
import concourse.tile as tile
from concourse import mybir
from concourse.bass_utils import run_bass_kernel_spmd

B, T, C, H, D = 2, 2048, 1024, 16, 64
HPC = 4          # heads per core
G = H // HPC     # head groups (cores per batch)
N_CORES = B * G
SCALE = 1.0 / np.sqrt(D)
P = 128
QT = 256         # q tile width
TT = T // QT     # q tiles (8)
NCC = C // P     # 128-deep contraction chunks (8)
VW = 66          # v cols per head: 64 + ones + pad
F32 = mybir.dt.float32
BF16 = mybir.dt.bfloat16
BF = ml_dtypes.bfloat16


def _tril_mask():
    p = np.arange(P)[:, None]
    f = np.arange(P)[None, :]
    return (p <= f).astype(np.float32)


# walrus in this toolchain can't encode >1 sem wait on one instruction
# ("Too many sync wait commands"); split excess waits onto preceding NoOps.
def _split_waits(nc, maxw=1):
    for f in nc.m.functions:
        for bb in f.blocks:
            out = []
            for inst in bb.instructions:
                si = getattr(inst, 'sync_info', None)
                if si is not None and si.on_wait and len(si.on_wait) > maxw:
                    waits = list(si.on_wait)
                    extra, keep = waits[:-maxw], waits[-maxw:]
                    for i in range(0, len(extra), maxw):
                        out.append(mybir.InstNoOp(
                            name=f"{inst.name}-wsplit{i}",
                            sync_info=mybir.SyncInfo(
                                on_wait=extra[i:i + maxw], on_update=[]),
                            bass_nofuse=True,
                            engine=inst.engine,
                        ))
                    inst.sync_info = mybir.SyncInfo(
                        on_wait=keep, on_update=list(si.on_update or []))
                out.append(inst)
            bb.instructions[:] = out


def build_nc(split=True):
    nc = bass.Bass()
    xT = nc.dram_tensor("xT", [C, T], BF16, kind="ExternalInput")
    wq = nc.dram_tensor("wq", [C, 256], BF16, kind="ExternalInput")
    wk = nc.dram_tensor("wk", [C, 256], BF16, kind="ExternalInput")
    wv = nc.dram_tensor("wv", [C, HPC * VW], BF16, kind="ExternalInput")
    bqk = nc.dram_tensor("bqk", [P, 4], F32, kind="ExternalInput")  # qA qB kA kB
    scs = nc.dram_tensor("scs", [P, 2 * T], BF16, kind="ExternalInput")  # sin|cos
    wout = nc.dram_tensor("wout", [256, C], BF16, kind="ExternalInput")
    y = nc.dram_tensor("y", [T, C], BF16, kind="ExternalOutput")
    masks_d = nc.inline_tensor(_tril_mask(), name="cmasks")

    with tile.TileContext(nc) as tc:
        with ExitStack() as ctx:
            # ---- resident pools ----
            wpool = ctx.enter_context(tc.tile_pool(name="wts", bufs=1))
            qkpool = ctx.enter_context(tc.tile_pool(name="qk", bufs=1))
            vpool = ctx.enter_context(tc.tile_pool(name="v", bufs=1))
            otpool = ctx.enter_context(tc.tile_pool(name="ot", bufs=1))

            bqk_sb = wpool.tile([P, 4], F32, tag="bqk")
            wq_sb = wpool.tile([P, NCC, 256], BF16, tag="wq")
            wk_sb = wpool.tile([P, NCC, 256], BF16, tag="wk")
            wv_sb = wpool.tile([P, NCC, HPC * VW], BF16, tag="wv")
            scs_sb = wpool.tile([P, 2 * T], BF16, tag="scs")
            sin4 = scs_sb[:, 0:T]
            cos4 = scs_sb[:, T:2 * T]
            wout_sb = wpool.tile([P, 2, C], BF16, tag="wout")
            masks_f = wpool.tile([P, P], F32, tag="masksf")
            masks_sb = wpool.tile([P, P], BF16, tag="masks")
            ones_f = wpool.tile([1, 64], F32, tag="onesf")
            nc.vector.memset(ones_f[:], 1.0)
            ones_sb = wpool.tile([1, 64], BF16, tag="ones")
            nc.vector.tensor_copy(ones_sb[:], ones_f[:])

            # q/k per head pair, rows = [A(h0) B(h0) A(h1) B(h1)] x 32
            # (A/B = rotary low/high halves) so one matmul contracts a
            # whole head (K=64)
            qp0 = qkpool.tile([P, T], BF16, tag="qp0")
            qp1 = qkpool.tile([P, T], BF16, tag="qp1")
            kp0 = qkpool.tile([P, T], BF16, tag="kp0")
            kp1 = qkpool.tile([P, T], BF16, tag="kp1")
            qk_tiles = [qp0, qp1, kp0, kp1]
            qp = [qp0, qp1]
            kp = [kp0, kp1]
            w_of = {0: wq_sb, 1: wq_sb, 2: wk_sb, 3: wk_sb}
            col_of = {0: 0, 1: 128, 2: 0, 3: 128}

            # V tiles [t-block, 4*66] (col 64 per head = ones, 65 = pad)
            NTB = T // P
            v_tiles = [vpool.tile([P, HPC * VW], BF16, tag=f"v{tb}",
                                  name=f"v{tb}")
                       for tb in range(NTB)]

            # O^T in SBUF: heads 0,1 stacked / heads 2,3 stacked
            ot_sb = [otpool.tile([P, T], BF16, tag=f"otsb{i}",
                                 name=f"otsb{i}")
                     for i in range(2)]

            with ExitStack() as stream:
                xpool = stream.enter_context(tc.tile_pool(name="x", bufs=2))
                rtmp = stream.enter_context(tc.tile_pool(name="rtmp", bufs=4))
                # pss tiles are [P, 2, 512] (2 banks) so the two concurrent
                # row-group-packed S matmuls (j=0/j=1) drain into DIFFERENT
                # banks — concurrent PE drains into one bank are a fatal
                # PSUM write collision on hardware.
                ps_s = stream.enter_context(
                    tc.tile_pool(name="pss", bufs=2, space="PSUM"))
                ps_ot = stream.enter_context(
                    tc.tile_pool(name="psot", bufs=2, space="PSUM"))
                ps_pr = stream.enter_context(
                    tc.tile_pool(name="pspr", bufs=2, space="PSUM"))
                espool = stream.enter_context(tc.tile_pool(name="es", bufs=6))
                dpool = stream.enter_context(tc.tile_pool(name="dv", bufs=2))
                opool = stream.enter_context(tc.tile_pool(name="osb", bufs=2))

                xt = {}

                def load_xt(tp):
                    # one DMA per contraction chunk so qkproj's cc-loop can
                    # start as soon as its chunk lands
                    t = xpool.tile([P, NCC, 2 * QT], BF16, tag="x",
                                   name=f"x_{tp}")
                    for cc in range(NCC):
                        nc.sync.dma_start(
                            t[:, cc, :],
                            xT[cc * P:(cc + 1) * P,
                               tp * 2 * QT:(tp + 1) * 2 * QT])
                    xt[tp] = t

                # q/k projection for one (jb, q-tile-pair): one 8-chunk
                # PSUM generation at N=512; evacuation folds the bias
                # (per-partition) and the bf16 downcast into one DVE op.
                def qkproj(jb, tp):
                    wsb, c0 = w_of[jb], col_of[jb]
                    dst = qk_tiles[jb][:, tp * 2 * QT:(tp + 1) * 2 * QT]
                    ps = ps_pr.tile([P, 512], F32, tag="pspr",
                                    name=f"qk{jb}_{tp}")
                    for cc in range(NCC):
                        nc.tensor.matmul(
                            ps[:], wsb[:, cc, c0:c0 + 128], xt[tp][:, cc, :],
                            start=(cc == 0), stop=(cc == NCC - 1))
                    nc.vector.tensor_scalar_add(dst, ps[:],
                                                bqk_sb[:, jb:jb + 1])

                # rope on the interleaved [A;B]-per-head layout:
                #   t' = t * cosF + swap32(t) * sinF'
                # where swap32 exchanges adjacent 32-partition blocks (DMA)
                # and sinF' = [-sin, +sin, -sin, +sin] (host-built sign).
                def rope(ti, tp):
                    t = qk_tiles[ti]
                    s = slice(tp * 2 * QT, (tp + 1) * 2 * QT)
                    u = rtmp.tile([P, 2 * QT], BF16, tag="ru", name="ru")
                    for blk in range(4):
                        r, rs = 32 * blk, 32 * (blk ^ 1)
                        nc.sync.dma_start(u[r:r + 32, :],
                                          t[rs:rs + 32, s])
                    t1 = rtmp.tile([P, 2 * QT], BF16, tag="r1", name="r1")
                    t2 = rtmp.tile([P, 2 * QT], BF16, tag="r2", name="r2")
                    nc.gpsimd.tensor_mul(t1[:], t[:, s], cos4[:, s])
                    nc.gpsimd.tensor_mul(t2[:], u[:], sin4[:, s])
                    nc.vector.tensor_add(t[:, s], t1[:], t2[:])

                def vproj(tb):
                    ps = ps_pr.tile([P, 512], F32, tag="pspr",
                                    name=f"psv{tb}")[:, 0:HPC * VW]
                    for cc in range(NCC):
                        nc.tensor.matmul(
                            ps, xt[tb // 4][:, cc, (tb % 4) * P:
                                            (tb % 4 + 1) * P],
                            wv_sb[:, cc, :],
                            start=(cc == 0), stop=(cc == NCC - 1))
                    nc.vector.tensor_copy(v_tiles[tb][:], ps)
                    ones_cols = v_tiles[tb].rearrange(
                        "p (h e) -> p h e", e=VW)[:, :, 64]
                    nc.vector.tensor_scalar_add(ones_cols, ones_cols, 1.0)

                # stage 1 of the divide: copy O^T psum (incl. denominator
                # row 64) to SBUF in bf16 — frees the psot banks for the
                # next tt — then kick off the denominator transpose DMA.
                def divides_a(tt, ot2, last=False):
                    # the last tile's divide chain rides the (idle by then)
                    # scalar DMA queue so the tail isn't stuck behind x/y DMAs
                    dq = nc.scalar if last else nc.sync
                    otf = []
                    dn = dpool.tile([P, 8], BF16, tag="dn", name="dn")
                    for pp in range(2):
                        of = dpool.tile([VW, 2, QT], BF16, tag=f"otf{pp}",
                                        name=f"otf{pp}")
                        nc.vector.tensor_copy(of[:], ot2[pp][:])
                        if not last:
                            dq.dma_start(
                                dn[:, 4 * pp:4 * pp + 4],
                                of[64:65, :, :].rearrange("a b c -> a (b c)"))
                        otf.append(of)
                    rr = dpool.tile([1, 2, 2, QT], BF16, tag="rr", name="rr")
                    with nc.allow_low_precision(
                            reason="softmax denom recip in bf16: 0.4% scale "
                                   "error, well inside the 2e-2 gate"):
                        if last:
                            # serial tail: skip the transpose round-trip,
                            # reciprocal straight on the [1, 512] denom rows
                            for pp in range(2):
                                nc.vector.reciprocal(rr[0:1, pp, :, :],
                                                     otf[pp][64:65, :, :])
                        else:
                            dnr = dpool.tile([P, 8], BF16, tag="dnr",
                                             name="dnr")
                            nc.vector.reciprocal(dnr[:], dn[:])
                            for pp in range(2):
                                dq.dma_start(
                                    rr[0:1, pp, :, :].rearrange(
                                        "a b c -> a (b c)"),
                                    dnr[:, 4 * pp:4 * pp + 4])
                    return (otf, rr)

                # stage 2: broadcast recip across 64 partitions via PE,
                # then ot_sb[h] = otf[h] * recip (DVE, bf16 2x mode).
                def divides_b(tt, pend):
                    otf, rr = pend
                    for pp in range(2):
                        rb = ps_pr.tile([P, 512], F32, tag="pspr",
                                        name=f"rb{pp}")[0:64, :]
                        nc.tensor.matmul(
                            rb, ones_sb[0:1, :],
                            rr[0:1, pp, :, :].rearrange("a b c -> a (b c)"),
                            start=True, stop=True)
                        rbs = dpool.tile([64, 2, QT], BF16, tag="rbs",
                                         name="rbs")
                        nc.vector.tensor_copy(
                            rbs[:], rb.rearrange("p (a b) -> p a b", a=2))
                        for j in range(2):
                            dst = ot_sb[pp][64 * j:64 * j + 64,
                                            tt * QT:(tt + 1) * QT]
                            nc.vector.tensor_mul(
                                dst, otf[pp][0:64, j, :], rbs[:, j, :])

                def outproj(tb):
                    o_sb = opool.tile([P, C], BF16, tag="osb", name="osb")
                    for nt in range(2):
                        ps = ps_pr.tile([P, 512], F32, tag="pspr",
                                        name=f"pso{tb}_{nt}")
                        for rc in range(2):
                            nc.tensor.matmul(
                                ps[:], ot_sb[rc][:, tb * P:(tb + 1) * P],
                                wout_sb[:, rc, nt * 512:(nt + 1) * 512],
                                start=(rc == 0), stop=(rc == 1))
                        nc.vector.tensor_copy(
                            o_sb[:, nt * 512:(nt + 1) * 512], ps[:])
                    nc.sync.dma_start(y[tb * P:(tb + 1) * P, :], o_sb[:])



                # ---- prologue: q-tile-pair 0's inputs and projections ----
                nc.scalar.dma_start(bqk_sb[:], bqk[:])
                load_xt(0)
                nc.scalar.dma_start(wq_sb[:],
                                  wq.rearrange("(o p) n -> p o n", p=P))
                nc.scalar.dma_start(wk_sb[:],
                                  wk.rearrange("(o p) n -> p o n", p=P))
                nc.scalar.dma_start(scs_sb[:], scs[:])
                nc.scalar.dma_start(wv_sb[:],
                                  wv.rearrange("(o p) n -> p o n", p=P))
                nc.scalar.dma_start(masks_f[:], masks_d[:])
                nc.vector.tensor_copy(masks_sb[:], masks_f[:])
                nc.scalar.dma_start(wout_sb[:],
                                  wout.rearrange("(o p) n -> p o n", p=P))
                for jb in range(4):
                    qkproj(jb, 0)
                for ti in (0, 2, 1, 3):
                    rope(ti, 0)
                for tb in range(4):
                    vproj(tb)

                # ---- streaming attention with injected work ----
                queue = []  # closures of next-tile + prev-tile work
                prev = None
                prev_out = []  # outproj closures of prev tile not yet run
                for tt in range(TT):
                    nk = 2 * tt + 2
                    if tt % 2 == 0 and tt // 2 + 1 < TT // 2:
                        ntp = tt // 2 + 1
                        load_xt(ntp)
                        queue += [lambda jb=jb, t=ntp: qkproj(jb, t)
                                  for jb in range(4)]
                        queue += [lambda ti=ti, t=ntp: rope(ti, t)
                                  for ti in (0, 2, 1, 3)]
                        queue += [lambda tb=tb: vproj(tb)
                                  for tb in range(4 * ntp, 4 * ntp + 4)]
                    ot2 = [ps_ot.tile([VW, 2, QT], F32, tag="psot",
                                      name=f"psot{pp}") for pp in range(2)]
                    es_prev = [None, None]
                    off_prev = [0, 0]
                    for kblk in range(nk):
                        off = max(0, (kblk - 2 * tt)) * P
                        ks = slice(kblk * P, (kblk + 1) * P)
                        qs = slice(tt * QT + off, (tt + 1) * QT)
                        for pp in range(2):
                            s2 = ps_s.tile([P, 2, 512], F32, tag="pss",
                                           name="pss")[:, :, 0:QT]
                            for j in range(2):
                                hs = slice(64 * j, 64 * j + 64)
                                nc.tensor.matmul(
                                    s2[:, j, off:], kp[pp][hs, ks],
                                    qp[pp][hs, qs],
                                    start=True, stop=True,
                                    tile_position=(64 * j, 0))
                            es2 = espool.tile([P, 2, QT], BF16, tag="es",
                                              name="es")
                            nc.scalar.activation(
                                es2[:, :, off:], s2[:, :, off:],
                                mybir.ActivationFunctionType.Exp, scale=SCALE)
                            if kblk >= 2 * tt:
                                nc.gpsimd.tensor_mul(
                                    es2[:, :, off:off + P],
                                    es2[:, :, off:off + P],
                                    masks_sb[:, None, :].to_broadcast(
                                        (P, 2, P)))
                            if kblk > 0:
                                for j in range(2):
                                    h = 2 * pp + j
                                    nc.tensor.matmul(
                                        ot2[pp][:, j, off_prev[pp]:],
                                        v_tiles[kblk - 1][:, VW * h:
                                                          VW * h + VW],
                                        es_prev[pp][:, j, off_prev[pp]:],
                                        start=(kblk == 1 and j == 0),
                                        stop=False)
                            es_prev[pp], off_prev[pp] = es2, off
                        # injected pipeline work from the previous q-tile
                        if prev is not None and kblk == 2:
                            divides_b(*prev)
                            prev = None
                        elif prev_out and prev is None and 3 <= kblk <= 4:
                            prev_out.pop(0)()
                        rounds_left = nk - 1 - kblk
                        if queue:
                            npop = (max(1, -(-len(queue) // rounds_left))
                                    if rounds_left > 0 else len(queue))
                            for _ in range(min(npop, len(queue))):
                                queue.pop(0)()
                    for pp in range(2):
                        for j in range(2):
                            h = 2 * pp + j
                            nc.tensor.matmul(
                                ot2[pp][:, j, off_prev[pp]:],
                                v_tiles[nk - 1][:, VW * h:VW * h + VW],
                                es_prev[pp][:, j, off_prev[pp]:],
                                start=False, stop=(j == 1))
                    # flush prev-tile leftovers (order: divides before outproj)
                    if prev is not None:
                        divides_b(*prev)
                        prev = None
                    for fn in prev_out:
                        fn()
                    prev_out = [lambda tb=tb: outproj(tb)
                                for tb in range(2 * tt, 2 * tt + 2)]
                    prev = (tt, divides_a(tt, ot2, last=(tt == TT - 1)))
                # tail
                if prev is not None:
                    divides_b(*prev)
                for fn in prev_out:
                    fn()

    if split:
        _split_waits(nc)
    return nc


def make_in_maps(x, rope_cache, Wqkv, bqkv, Wout, bout):
    """Host-side shard prep. Returns list of 8 in_maps (core = 4*b + g)."""
    x = np.asarray(x, np.float32)
    rope_cache = np.asarray(rope_cache, np.float32)
    Wqkv = np.asarray(Wqkv, np.float32)
    bqkv = np.asarray(bqkv, np.float32)
    Wout = np.asarray(Wout, np.float32)

    # rotary-half permutation within a head: [evens, odds]
    perm = np.concatenate([np.arange(0, D, 2), np.arange(1, D, 2)])
    sin = rope_cache[:, 0::2].T.copy()   # [32, T]
    cos = rope_cache[:, 1::2].T.copy()
    # signed sin for the swap32 rope: rows [-s, +s, -s, +s]; cos tiled 4x
    sinF = np.concatenate([-sin, sin, -sin, sin], axis=0)
    cosF = np.tile(cos, (4, 1))
    scs = np.concatenate([sinF, cosF], axis=1).astype(BF)  # [128, 2T]

    xT = [np.ascontiguousarray(x[b].T).astype(BF) for b in range(B)]

    in_maps = []
    for core in range(N_CORES):
        b, g = divmod(core, G)
        heads = range(HPC * g, HPC * g + HPC)
        # per-head interleave: [A(h0) B(h0) A(h1) B(h1)] for the pp0 tile
        # (heads 0,1 of the core) then the same for pp1 (heads 2,3)
        qcols, kcols, vcols = [], [], []
        for h in heads:
            dd = h * D + perm  # [A(32), B(32)] for this head
            qcols.extend(0 * C + dd)
            kcols.extend(1 * C + dd)
        for h in heads:
            vcols.extend(2 * C + h * D + np.arange(D))
        qcols = np.asarray(qcols)
        kcols = np.asarray(kcols)
        vcols = np.asarray(vcols)
        wq_c = np.ascontiguousarray(Wqkv[:, qcols]).astype(BF)
        wk_c = np.ascontiguousarray(Wqkv[:, kcols]).astype(BF)
        wv_c = np.zeros((C, HPC * VW), np.float32)
        vv = Wqkv[:, vcols]
        for h in range(HPC):
            wv_c[:, VW * h:VW * h + 64] = vv[:, 64 * h:64 * h + 64]
        bqk_c = np.stack([bqkv[qcols[:128]], bqkv[qcols[128:]],
                          bqkv[kcols[:128]], bqkv[kcols[128:]]], axis=1)
        rows = np.arange(HPC * g * D, (HPC * g + HPC) * D)
        wout_c = np.ascontiguousarray(Wout[rows, :]).astype(BF)
        in_maps.append({
            "xT": xT[b], "wq": wq_c, "wk": wk_c,
            "wv": np.ascontiguousarray(wv_c.astype(BF)),
            "bqk": np.ascontiguousarray(bqk_c.astype(np.float32)),
            "scs": scs, "wout": wout_c,
        })
    return in_maps


_NC_CACHE = None


def _get_nc():
    global _NC_CACHE
    if _NC_CACHE is None:
        _NC_CACHE = build_nc()
    return _NC_CACHE


def run(inputs, trace=False):
    nc = _get_nc()
    in_maps = make_in_maps(**inputs)
    res = run_bass_kernel_spmd(nc, in_maps, list(range(N_CORES)), trace=trace)
    Wqkv = np.asarray(inputs["Wqkv"], np.float32)
    bqkv = np.asarray(inputs["bqkv"], np.float32)
    Wout = np.asarray(inputs["Wout"], np.float32)
    bout = np.asarray(inputs["bout"], np.float32)
    bvW = bqkv[2 * C:3 * C] @ Wout            # v-bias through out-proj
    out = np.zeros((B, T, C), np.float32)
    for core in range(N_CORES):
        out[core // G] += np.asarray(res.results[core]["y"], np.float32)
    out += (bvW + bout)[None, None, :]
    return out, res


def kernel(**inputs):
    out, _ = run(inputs)
    return out
